# revision 1
# baseline (speedup 1.0000x reference)
"""PointerGenerator kernel for 8 TRN2 NeuronCores (Bass/Tile).

Strategy (vocab-sharded softmax):
  - The dominant cost is scores = hf @ Wv with hf (2048, 512) and Wv
    (512, 32000).  Wv is sharded column-wise: core c owns vocab columns
    [4000c, 4000c+4000) and computes scores/exp/probs for ALL 2048 rows
    of its slice.  Row sums for the softmax are combined across cores
    with small pipelined AllReduces (8 row-groups of 256 rows).
  - The copy-distribution matmuls (source/target attention one-hot
    scatter) are row-sharded: core c owns rows [256c, 256c+256), which
    all live in batch c//2.
  - hiddens / attentions are pre-transposed on the host so every matmul
    operand loads with the contraction dim on partitions.
  - predictions: each core emits the max of its scaled vocab slice per
    row (fused into the pipeline) plus the max of its copy block; the
    host picks the winning region per row and looks up the index inside
    that region only (tiny scans of data we already output).

Precision modes for the big matmul (MODE below):
  f32   : exact fp32 (PE runs fp32 at 1/4 rate)
  f32r  : tf32-like, full PE rate, ~1e-3 relative error on scores
  f16x3 : hf and Wv split into fp16 hi+lo; 3 matmuls (hh, hl, lh) give
          ~1e-7 relative error at 3/4 of the bf16 rate.
The p-switch matmul and the copy matmuls are always exact fp32 (they
decide most argmaxes).
"""

import json
import os
import sys
import types

import numpy as np

sys.path.insert(0, "/opt/trn_rl_repo")

import concourse.bass as bass
import concourse.mybir as mybir
import concourse.tile as tile
from concourse.bass_utils import run_bass_kernel_spmd

# ----------------------------------------------------------------------------
# Toolchain fixups
# ----------------------------------------------------------------------------

_MAX_WAITS = 1  # this walrus build accepts at most one sync-wait per inst


def _split_waits(bir: dict) -> dict:
    """Hoist extra sync-waits onto standalone EventSemaphore instructions
    (same engine, inserted just before — program order preserves semantics).
    The installed walrus rejects >1 wait command per instruction."""
    ctr = 0
    for f in bir.get("functions", []):
        for blk in f.get("blocks", []):
            insts = blk.get("instructions", [])
            out = []
            changed = False
            for ins in insts:
                si = ins.get("sync_info")
                waits = (si or {}).get("on_wait") or []
                if len(waits) > _MAX_WAITS:
                    hoist, keep = waits[:-_MAX_WAITS], waits[-_MAX_WAITS:]
                    for w in hoist:
                        ctr += 1
                        out.append({
                            "debug": ins.get("debug", 0),
                            "engine": ins["engine"],
                            "ins": [],
                            "outs": [],
                            "name": f"wsplit-{ctr}",
                            "opcode": "EventSemaphore",
                            "sync_info": {"on_update": [], "on_wait": [w]},
                        })
                    si["on_wait"] = keep
                    changed = True
                out.append(ins)
            if changed:
                blk["instructions"] = out
    return bir


def _install_fixups():
    if getattr(bass.Bass, "_wait_split_patched", False):
        return
    orig = bass.Bass.to_json_bytes

    def to_json_bytes(self):
        bir = json.loads(orig(self))
        return json.dumps(_split_waits(bir)).encode()

    bass.Bass.to_json_bytes = to_json_bytes
    bass.Bass._wait_split_patched = True


def _install_ntff_hook():
    """Provide antenv.axon_hooks so trace=True can capture NTFF profiles."""
    if "antenv.axon_hooks" in sys.modules:
        return
    try:
        if "/root/.axon_site" not in sys.path:
            sys.path.insert(0, "/root/.axon_site")
        from trn_agent_boot.trn_boot import _ntff_profile_via_ctypes

        hook = _ntff_profile_via_ctypes("/opt/axon/libaxon_pjrt.so")
    except Exception:
        hook = None
    mod = types.ModuleType("antenv.axon_hooks")
    mod.get_axon_ntff_profile_hook = lambda: hook
    mod.set_axon_ntff_profile_hook = lambda h: None
    sys.modules["antenv.axon_hooks"] = mod


_install_fixups()
_install_ntff_hook()

# ----------------------------------------------------------------------------
# Problem constants (hardcoded per spec)
# ----------------------------------------------------------------------------

B, T, H, V = 4, 512, 512, 32000
S, SDV, TDV = 512, 300, 512
R = B * T                      # 2048 rows
N_CORES = 8
VS = V // N_CORES              # 4000 vocab cols per core
RPC = R // N_CORES             # 256 rows per core (copy path)
M_TILES = R // 128             # 16
KT = H // 128                  # 4
MT_PER_G = 2                   # m-tiles per allreduce group
GROUPS = M_TILES // MT_PER_G   # 8
CHUNKS = (1536, 1536, 928)     # psum chunking of the 4000-wide slice
NEG = -1.0e30

MODE = os.environ.get("PG_MODE", "f32r")  # f32 | f32r | f16x3

F32 = mybir.dt.float32
F32R = mybir.dt.float32r
F16 = mybir.dt.float16
AX = mybir.AxisListType
OP = mybir.AluOpType
AF = mybir.ActivationFunctionType

_KERNEL_CACHE = {}


def _build(mode: str):
    nc = bass.Bass(num_devices=N_CORES)

    mm_dt = {"f32": F32, "f32r": F32R, "f16x3": F16}[mode]

    # ---------------- inputs ----------------
    # hfT32: hf.T in f32, used for the p-switch matmul (and the big matmul
    # in f32 mode).
    hfT32 = nc.dram_tensor("hfT32", [H, R], F32, kind="ExternalInput")
    if mode == "f32r":
        hfT_m = nc.dram_tensor("hfT", [H, R], F32R, kind="ExternalInput")
        wv_m = nc.dram_tensor("wv", [H, VS], F32R, kind="ExternalInput")
    elif mode == "f16x3":
        hfT_m = nc.dram_tensor("hfT", [H, R], F16, kind="ExternalInput")
        hfT_lo = nc.dram_tensor("hfT_lo", [H, R], F16, kind="ExternalInput")
        wv_m = nc.dram_tensor("wv", [H, VS], F16, kind="ExternalInput")
        wv_lo = nc.dram_tensor("wv_lo", [H, VS], F16, kind="ExternalInput")
    else:
        hfT_m = hfT32
        wv_m = nc.dram_tensor("wv", [H, VS], F32, kind="ExternalInput")

    hfT_own = nc.dram_tensor("hfT_own", [H, RPC], F32, kind="ExternalInput")
    wp = nc.dram_tensor("wp", [H, 3], F32, kind="ExternalInput")
    bp_rep = nc.dram_tensor("bp_rep", [128, 3], F32, kind="ExternalInput")
    maskbias = nc.dram_tensor("maskbias", [128, 1], F32, kind="ExternalInput")
    srcT = nc.dram_tensor("srcT", [S, RPC], F32, kind="ExternalInput")
    smap = nc.dram_tensor("smap", [S, SDV], F32, kind="ExternalInput")
    tgtT = nc.dram_tensor("tgtT", [TDV, RPC], F32, kind="ExternalInput")
    tmap = nc.dram_tensor("tmap", [TDV, TDV], F32, kind="ExternalInput")

    # ---------------- outputs ----------------
    out_vocab = nc.dram_tensor("out_vocab", [R, VS], F32, kind="ExternalOutput")
    out_copy = nc.dram_tensor("out_copy", [RPC, SDV + TDV], F32,
                              kind="ExternalOutput")
    vmax_o = nc.dram_tensor("vmax", [R, 1], F32, kind="ExternalOutput")
    cmax_o = nc.dram_tensor("cmax", [RPC, 1], F32, kind="ExternalOutput")
    p_o = nc.dram_tensor("p_out", [R, 3], F32, kind="ExternalOutput")

    with tile.TileContext(nc) as tc:
        with (
            tc.tile_pool(name="wvp", bufs=1) as wvp,
            tc.tile_pool(name="hfp", bufs=4) as hfp,
            tc.tile_pool(name="ep", bufs=2 * MT_PER_G) as epool,
            tc.tile_pool(name="small", bufs=4) as smallp,
            tc.tile_pool(name="cst", bufs=1) as cstp,
            tc.tile_pool(name="accp", bufs=2 * MT_PER_G) as accp,
            tc.tile_pool(name="psA", bufs=2, space="PSUM") as psA,   # 2x3 banks
            tc.tile_pool(name="psB", bufs=2, space="PSUM") as psB,   # 2x1 bank
            tc.tile_pool(name="dram", bufs=2 * GROUPS + 2, space="DRAM") as dp,
        ):
            # --- dummy collective: absorbs core launch skew + CC init ---
            warm_in = dp.tile([1, 8], F32)
            warm_out = dp.tile([1, 8], F32)
            nc.gpsimd.collective_compute(
                "AllReduce", OP.add,
                replica_groups=[list(range(N_CORES))],
                ins=[warm_in[:].opt()], outs=[warm_out[:].opt()],
            )

            # --- resident weights / constants ---
            wv_t = wvp.tile([128, KT, VS], mm_dt, tag="wv")
            nc.sync.dma_start(wv_t[:], wv_m.rearrange("(kt p) v -> p kt v", p=128))
            if mode == "f16x3":
                wv_lo_t = wvp.tile([128, KT, VS], F16, tag="wvlo")
                nc.sync.dma_start(
                    wv_lo_t[:], wv_lo.rearrange("(kt p) v -> p kt v", p=128))

            wp_t = cstp.tile([128, KT, 3], F32, tag="wp")
            nc.sync.dma_start(wp_t[:], wp.rearrange("(kt p) n -> p kt n", p=128))
            bp_t = cstp.tile([128, 3], F32, tag="bp")
            nc.sync.dma_start(bp_t[:], bp_rep[:, :])
            mask_t = cstp.tile([128, 1], F32, tag="mask")
            nc.sync.dma_start(mask_t[:], maskbias[:, :])

            # --- copy path (own 256 rows), exact fp32 ---
            srcT_t = cstp.tile([128, KT, RPC], F32, tag="srcT")
            nc.sync.dma_start(srcT_t[:], srcT.rearrange("(kt p) r -> p kt r", p=128))
            smap_t = cstp.tile([128, KT, SDV], F32, tag="smap")
            nc.sync.dma_start(smap_t[:], smap.rearrange("(kt p) v -> p kt v", p=128))
            tgtT_t = cstp.tile([128, KT, RPC], F32, tag="tgtT")
            nc.sync.dma_start(tgtT_t[:], tgtT.rearrange("(kt p) r -> p kt r", p=128))
            tmap_t = cstp.tile([128, KT, TDV], F32, tag="tmap")
            nc.sync.dma_start(tmap_t[:], tmap.rearrange("(kt p) v -> p kt v", p=128))
            hfo_t = cstp.tile([128, KT, RPC], F32, tag="hfo")
            nc.sync.dma_start(hfo_t[:], hfT_own.rearrange("(kt p) r -> p kt r", p=128))

            for mo in range(RPC // 128):  # 2 own m-tiles
                rs = slice(mo * 128, (mo + 1) * 128)
                # p for own rows (identical math to the big-loop p below)
                ps_p = psB.tile([128, 3], F32, tag="psB")
                for ki in range(KT):
                    nc.tensor.matmul(ps_p[:], hfo_t[:, ki, rs], wp_t[:, ki, :],
                                     start=(ki == 0), stop=(ki == KT - 1))
                nc.vector.tensor_add(ps_p[:], ps_p[:], bp_t[:])
                p_own = smallp.tile([128, 3], F32, tag="p_own")
                psum_own = smallp.tile([128, 1], F32, tag="psum_own")
                nc.scalar.activation(p_own[:], ps_p[:], AF.Exp,
                                     accum_out=psum_own[:])
                prcp = smallp.tile([128, 1], F32, tag="prcp")
                nc.vector.reciprocal(prcp[:], psum_own[:])
                nc.vector.tensor_scalar(p_own[:], p_own[:], prcp[:], None,
                                        op0=OP.mult)

                # copy-source block: (A_chunk @ smap) * p_cs
                ps_cs = psB.tile([128, SDV], F32, tag="psB")
                for ki in range(KT):
                    nc.tensor.matmul(ps_cs[:], srcT_t[:, ki, rs],
                                     smap_t[:, ki, :],
                                     start=(ki == 0), stop=(ki == KT - 1))
                o_cs = smallp.tile([128, SDV], F32, tag="o_cs")
                nc.vector.tensor_scalar(o_cs[:], ps_cs[:], p_own[:, 0:1], None,
                                        op0=OP.mult)
                # copy-target block: (Tattn_chunk @ tmap) * p_ct
                ps_ct = psB.tile([128, TDV], F32, tag="psB")
                for ki in range(KT):
                    nc.tensor.matmul(ps_ct[:], tgtT_t[:, ki, rs],
                                     tmap_t[:, ki, :],
                                     start=(ki == 0), stop=(ki == KT - 1))
                o_ct = smallp.tile([128, TDV], F32, tag="o_ct")
                nc.vector.tensor_scalar(o_ct[:], ps_ct[:], p_own[:, 1:2], None,
                                        op0=OP.mult)

                cmx = smallp.tile([128, 2], F32, tag="cmx")
                nc.vector.tensor_reduce(cmx[:, 0:1], o_cs[:], axis=AX.X, op=OP.max)
                nc.vector.tensor_reduce(cmx[:, 1:2], o_ct[:], axis=AX.X, op=OP.max)
                cmx1 = smallp.tile([128, 1], F32, tag="cmx1")
                nc.vector.tensor_reduce(cmx1[:], cmx[:], axis=AX.X, op=OP.max)

                nc.sync.dma_start(out_copy[rs, 0:SDV], o_cs[:])
                nc.sync.dma_start(out_copy[rs, SDV:SDV + TDV], o_ct[:])
                nc.sync.dma_start(cmax_o[rs, :], cmx1[:])

            # --- main vocab loop, grouped for pipelined allreduces ---
            for g in range(GROUPS):
                sum_ts = []
                e_ts = []
                pg_ts = []
                for t_ in range(MT_PER_G):
                    mi = g * MT_PER_G + t_
                    ms = slice(mi * 128, (mi + 1) * 128)

                    hf32_t = hfp.tile([128, KT, 128], F32, tag="hf32")
                    nc.sync.dma_start(
                        hf32_t[:],
                        hfT32.rearrange("(kt p) r -> p kt r", p=128)[:, :, ms])
                    if mode == "f32":
                        hfm_t = hf32_t
                    else:
                        hfm_t = hfp.tile([128, KT, 128], mm_dt, tag="hfm")
                        nc.sync.dma_start(
                            hfm_t[:],
                            hfT_m.rearrange("(kt p) r -> p kt r", p=128)[:, :, ms])
                        if mode == "f16x3":
                            hfl_t = hfp.tile([128, KT, 128], F16, tag="hfl")
                            nc.sync.dma_start(
                                hfl_t[:],
                                hfT_lo.rearrange("(kt p) r -> p kt r", p=128)[:, :, ms])

                    # p switch probs (exact f32) for this m-tile
                    ps_p = psB.tile([128, 3], F32, tag="psB")
                    for ki in range(KT):
                        nc.tensor.matmul(ps_p[:], hf32_t[:, ki, :], wp_t[:, ki, :],
                                         start=(ki == 0), stop=(ki == KT - 1))
                    nc.vector.tensor_add(ps_p[:], ps_p[:], bp_t[:])
                    p_t = smallp.tile([128, 3], F32, tag="p_t")
                    psum_t = smallp.tile([128, 1], F32, tag="psum_t")
                    nc.scalar.activation(p_t[:], ps_p[:], AF.Exp,
                                         accum_out=psum_t[:])
                    prcp = smallp.tile([128, 1], F32, tag="prcp2")
                    nc.vector.reciprocal(prcp[:], psum_t[:])
                    nc.vector.tensor_scalar(p_t[:], p_t[:], prcp[:], None,
                                            op0=OP.mult)
                    nc.sync.dma_start(p_o[ms, :], p_t[:])
                    pg_ts.append(p_t)

                    # big matmul in psum chunks; exp+rowsum fused per chunk
                    e_t = epool.tile([128, VS], F32, tag="e")
                    acc_t = accp.tile([128, 3], F32, tag="acc")
                    col = 0
                    for ci, cw in enumerate(CHUNKS):
                        ps_c = psA.tile([128, cw], F32, tag="psA")
                        for n0 in range(0, cw, 512):
                            nw = min(512, cw - n0)
                            for ki in range(KT):
                                first = ki == 0
                                last = ki == KT - 1
                                if mode == "f16x3":
                                    nc.tensor.matmul(
                                        ps_c[:, n0:n0 + nw], hfm_t[:, ki, :],
                                        wv_t[:, ki, col + n0:col + n0 + nw],
                                        start=first, stop=False)
                                    nc.tensor.matmul(
                                        ps_c[:, n0:n0 + nw], hfm_t[:, ki, :],
                                        wv_lo_t[:, ki, col + n0:col + n0 + nw],
                                        start=False, stop=False)
                                    nc.tensor.matmul(
                                        ps_c[:, n0:n0 + nw], hfl_t[:, ki, :],
                                        wv_t[:, ki, col + n0:col + n0 + nw],
                                        start=False, stop=last)
                                else:
                                    nc.tensor.matmul(
                                        ps_c[:, n0:n0 + nw], hfm_t[:, ki, :],
                                        wv_t[:, ki, col + n0:col + n0 + nw],
                                        start=first, stop=last)
                        if ci == 0:
                            # global vocab col 0 pad mask (-1e30 on core 0)
                            nc.vector.tensor_add(ps_c[:, 0:1], ps_c[:, 0:1],
                                                 mask_t[:])
                        nc.scalar.activation(e_t[:, col:col + cw], ps_c[:],
                                             AF.Exp,
                                             accum_out=acc_t[:, ci:ci + 1])
                        col += cw

                    sumloc = smallp.tile([128, 1], F32, tag="sumloc")
                    nc.vector.tensor_reduce(sumloc[:], acc_t[:], axis=AX.X,
                                            op=OP.add)
                    sum_ts.append(sumloc)
                    e_ts.append(e_t)

                # allreduce this group's row sums
                bin_ = dp.tile([MT_PER_G, 128], F32)
                bout = dp.tile([MT_PER_G, 128], F32)
                for t_ in range(MT_PER_G):
                    nc.sync.dma_start(bin_[t_:t_ + 1, :], sum_ts[t_][:])
                nc.gpsimd.collective_compute(
                    "AllReduce", OP.add,
                    replica_groups=[list(range(N_CORES))],
                    ins=[bin_[:].opt()], outs=[bout[:].opt()],
                )

                for t_ in range(MT_PER_G):
                    mi = g * MT_PER_G + t_
                    ms = slice(mi * 128, (mi + 1) * 128)
                    sf = smallp.tile([128, 1], F32, tag="sf")
                    nc.sync.dma_start(sf[:], bout[t_:t_ + 1, :])
                    rcp = smallp.tile([128, 1], F32, tag="rcp")
                    nc.vector.reciprocal(rcp[:], sf[:])
                    scale = smallp.tile([128, 1], F32, tag="scale")
                    nc.vector.tensor_mul(scale[:], rcp[:], pg_ts[t_][:, 2:3])

                    e_t = e_ts[t_]
                    vmx = smallp.tile([128, 1], F32, tag="vmx")
                    if t_ % 2 == 0:
                        nc.vector.tensor_scalar(e_t[:], e_t[:], scale[:], None,
                                                op0=OP.mult)
                    else:
                        nc.scalar.activation(e_t[:], e_t[:], AF.Copy,
                                             scale=scale[:])
                    nc.vector.tensor_reduce(vmx[:], e_t[:], axis=AX.X, op=OP.max)
                    nc.sync.dma_start(out_vocab[ms, :], e_t[:])
                    nc.sync.dma_start(vmax_o[ms, :], vmx[:])

    return nc


def _get_kernel(mode: str):
    if mode not in _KERNEL_CACHE:
        _KERNEL_CACHE[mode] = _build(mode)
    return _KERNEL_CACHE[mode]


# ----------------------------------------------------------------------------
# Host side
# ----------------------------------------------------------------------------

def _f16_split(a: np.ndarray):
    hi = a.astype(np.float16)
    lo = (a - hi.astype(np.float32)).astype(np.float16)
    return hi, lo


def _prepare_inputs(mode, hiddens, Wp, bp, Wv, bv, source_attentions,
                    source_attention_maps, target_attentions,
                    target_attention_maps):
    hf = np.ascontiguousarray(hiddens.reshape(R, H))
    hfT = np.ascontiguousarray(hf.T)                       # (H, R) f32
    wp_f = np.ascontiguousarray(Wp.astype(np.float32))
    bp_rep = np.broadcast_to(bp.astype(np.float32), (128, 3)).copy()

    if mode == "f16x3":
        hfT_hi, hfT_lo = _f16_split(hfT)
        wv_hi, wv_lo = _f16_split(Wv)

    in_maps = []
    for c in range(N_CORES):
        b = c // (N_CORES // B)
        toff = (c % (N_CORES // B)) * RPC
        rs = slice(c * RPC, (c + 1) * RPC)
        vs = slice(c * VS, (c + 1) * VS)

        mb = np.zeros((128, 1), np.float32)
        if c == 0:
            mb[:] = NEG

        m = {
            "hfT32": hfT,
            "hfT_own": np.ascontiguousarray(hfT[:, rs]),
            "wp": wp_f,
            "bp_rep": bp_rep,
            "maskbias": mb,
            "srcT": np.ascontiguousarray(
                source_attentions[b, toff:toff + RPC, :].T),
            "smap": np.ascontiguousarray(source_attention_maps[b]),
            "tgtT": np.ascontiguousarray(
                target_attentions[b, toff:toff + RPC, :].T),
            "tmap": np.ascontiguousarray(target_attention_maps[b]),
        }
        if mode == "f32r":
            m["hfT"] = hfT
            m["wv"] = np.ascontiguousarray(Wv[:, vs])
        elif mode == "f16x3":
            m["hfT"] = hfT_hi
            m["hfT_lo"] = hfT_lo
            m["wv"] = np.ascontiguousarray(wv_hi[:, vs])
            m["wv_lo"] = np.ascontiguousarray(wv_lo[:, vs])
        else:
            m["wv"] = np.ascontiguousarray(Wv[:, vs])
        in_maps.append(m)
    return in_maps


def _assemble(results, bv):
    VT = V + SDV + TDV
    probs = np.empty((R, VT), np.float32)
    vmax = np.empty((R, N_CORES), np.float32)
    copy_blk = np.empty((R, SDV + TDV), np.float32)
    cmax = np.empty((R,), np.float32)
    for c in range(N_CORES):
        r = results[c]
        probs[:, c * VS:(c + 1) * VS] = r["out_vocab"]
        vmax[:, c] = r["vmax"][:, 0]
        rs = slice(c * RPC, (c + 1) * RPC)
        copy_blk[rs] = r["out_copy"]
        cmax[rs] = r["cmax"][:, 0]
    probs[:, V:] = copy_blk

    if np.any(bv):
        # setup_inputs always produces bv == 0; exact-general fallback for
        # nonzero bv recomputes the vocab renormalization on the host.
        p_out = results[0]["p_out"]
        p_gen = p_out[:, 2:3]
        e_rel = probs[:, :V] / np.where(p_gen == 0, 1.0, p_gen)
        e_rel = e_rel * np.exp(bv.astype(np.float32))[None, :]
        probs[:, :V] = e_rel / e_rel.sum(axis=1, keepdims=True) * p_gen
        vocab_max = probs[:, :V].max(axis=1)
        best = np.where(cmax >= vocab_max, 1, 0)
        preds = np.empty((R,), np.int64)
        for i in range(R):
            preds[i] = int(np.argmax(probs[i]))
        return probs, preds.astype(np.int32)

    # region winner per row: [vocab shard 0..7, copy]; first max wins so the
    # natural global-column order is preserved on exact ties.
    cand = np.concatenate([vmax, cmax[:, None]], axis=1)   # (R, 9)
    best = np.argmax(cand, axis=1)

    preds = np.empty((R,), np.int64)
    copy_idx = np.argmax(copy_blk, axis=1)                 # cheap, all rows
    for i in range(R):
        c = best[i]
        if c == N_CORES:
            preds[i] = V + copy_idx[i]
        else:
            preds[i] = c * VS + int(np.argmax(probs[i, c * VS:(c + 1) * VS]))
    return probs, preds


def kernel(hiddens, Wp, bp, Wv, bv, source_attentions, source_attention_maps,
           target_attentions, target_attention_maps):
    mode = MODE
    hiddens = np.asarray(hiddens, np.float32)
    Wp = np.asarray(Wp, np.float32)
    bp = np.asarray(bp, np.float32)
    Wv = np.asarray(Wv, np.float32)
    bv = np.asarray(bv, np.float32)
    source_attentions = np.asarray(source_attentions, np.float32)
    source_attention_maps = np.asarray(source_attention_maps, np.float32)
    target_attentions = np.asarray(target_attentions, np.float32)
    target_attention_maps = np.asarray(target_attention_maps, np.float32)

    nc = _get_kernel(mode)
    in_maps = _prepare_inputs(mode, hiddens, Wp, bp, Wv, bv,
                              source_attentions, source_attention_maps,
                              target_attentions, target_attention_maps)
    res = run_bass_kernel_spmd(
        nc, in_maps, core_ids=list(range(N_CORES)),
        trace=bool(int(os.environ.get("PG_TRACE", "0"))),
        tmpdir=os.environ.get("PG_TMPDIR"),
    )
    kernel.last_result = res

    probs, preds = _assemble(res.results, bv)
    probs = probs.reshape(B, T, V + SDV + TDV)
    preds = np.asarray(preds, np.int32).reshape(B, T)
    return probs, preds


# revision 7
# speedup vs baseline: 1.4797x; 1.4797x over previous
"""PointerGenerator kernel for 8 TRN2 NeuronCores (Bass/Tile).

Strategy (vocab-sharded softmax):
  - The dominant cost is scores = hf @ Wv with hf (2048, 512) and Wv
    (512, 32000).  Wv is sharded column-wise: core c owns vocab columns
    [4000c, 4000c+4000) and computes scores/exp/probs for ALL 2048 rows
    of its slice.  Row sums for the softmax are combined across cores
    with small pipelined AllReduces (8 row-groups of 256 rows).
  - The copy-distribution matmuls (source/target attention one-hot
    scatter) are row-sharded: core c owns rows [256c, 256c+256), which
    all live in batch c//2.
  - hiddens / attentions are pre-transposed on the host so every matmul
    operand loads with the contraction dim on partitions.
  - predictions: each core emits the max of its scaled vocab slice per
    row (fused into the pipeline) plus the max of its copy block; the
    host picks the winning region per row and looks up the index inside
    that region only (tiny scans of data we already output).

Precision modes for the big matmul (MODE below):
  f32   : exact fp32 (PE runs fp32 at 1/4 rate)
  f32r  : tf32-like, full PE rate, ~1e-3 relative error on scores
  f16x3 : hf and Wv split into fp16 hi+lo; 3 matmuls (hh, hl, lh) give
          ~1e-7 relative error at 3/4 of the bf16 rate.
The p-switch matmul and the copy matmuls are always exact fp32 (they
decide most argmaxes).
"""

import json
import os
import sys
import types

import numpy as np

sys.path.insert(0, "/opt/trn_rl_repo")

import concourse.bass as bass
import concourse.mybir as mybir
import concourse.tile as tile
from concourse.bass_utils import run_bass_kernel_spmd

# ----------------------------------------------------------------------------
# Toolchain fixups
# ----------------------------------------------------------------------------

_MAX_WAITS = 1  # this walrus build accepts at most one sync-wait per inst


def _split_waits(bir: dict) -> dict:
    """Hoist extra sync-waits onto standalone EventSemaphore instructions
    (same engine, inserted just before — program order preserves semantics).
    The installed walrus rejects >1 wait command per instruction."""
    ctr = 0
    for f in bir.get("functions", []):
        for blk in f.get("blocks", []):
            insts = blk.get("instructions", [])
            out = []
            changed = False
            for ins in insts:
                si = ins.get("sync_info")
                waits = (si or {}).get("on_wait") or []
                if len(waits) > _MAX_WAITS:
                    hoist, keep = waits[:-_MAX_WAITS], waits[-_MAX_WAITS:]
                    for w in hoist:
                        ctr += 1
                        out.append({
                            "debug": ins.get("debug", 0),
                            "engine": ins["engine"],
                            "ins": [],
                            "outs": [],
                            "name": f"wsplit-{ctr}",
                            "opcode": "EventSemaphore",
                            "sync_info": {"on_update": [], "on_wait": [w]},
                        })
                    si["on_wait"] = keep
                    changed = True
                out.append(ins)
            if changed:
                blk["instructions"] = out
    return bir


def _install_fixups():
    if getattr(bass.Bass, "_wait_split_patched", False):
        return
    orig = bass.Bass.to_json_bytes

    def to_json_bytes(self):
        bir = json.loads(orig(self))
        return json.dumps(_split_waits(bir)).encode()

    bass.Bass.to_json_bytes = to_json_bytes
    bass.Bass._wait_split_patched = True


def _install_ntff_hook():
    """Provide antenv.axon_hooks so trace=True can capture NTFF profiles."""
    if "antenv.axon_hooks" in sys.modules:
        return
    try:
        if "/root/.axon_site" not in sys.path:
            sys.path.insert(0, "/root/.axon_site")
        from trn_agent_boot.trn_boot import _ntff_profile_via_ctypes

        hook = _ntff_profile_via_ctypes("/opt/axon/libaxon_pjrt.so")
    except Exception:
        hook = None
    mod = types.ModuleType("antenv.axon_hooks")
    mod.get_axon_ntff_profile_hook = lambda: hook
    mod.set_axon_ntff_profile_hook = lambda h: None
    sys.modules["antenv.axon_hooks"] = mod


_install_fixups()
_install_ntff_hook()

# ----------------------------------------------------------------------------
# Problem constants (hardcoded per spec)
# ----------------------------------------------------------------------------

B, T, H, V = 4, 512, 512, 32000
S, SDV, TDV = 512, 300, 512
R = B * T                      # 2048 rows
N_CORES = 8
VS = V // N_CORES              # 4000 vocab cols per core
RPC = R // N_CORES             # 256 rows per core (copy path)
M_TILES = R // 128             # 16
KT = H // 128                  # 4
MT_PER_G = 4                   # m-tiles per allreduce group
GROUPS = M_TILES // MT_PER_G   # 4
CHUNKS = (1536, 1536, 928)     # psum chunking of the 4000-wide slice
NEG = -1.0e30

MODE = os.environ.get("PG_MODE", "f32r")  # f32 | f32r | f16x3

F32 = mybir.dt.float32
F32R = mybir.dt.float32r
F16 = mybir.dt.float16
AX = mybir.AxisListType
OP = mybir.AluOpType
AF = mybir.ActivationFunctionType

_KERNEL_CACHE = {}


def _build(mode: str):
    nc = bass.Bass(num_devices=N_CORES)

    mm_dt = {"f32": F32, "f32r": F32R, "f16x3": F16}[mode]

    # ---------------- inputs ----------------
    # hfT32: hf.T in f32, used for the p-switch matmul (and the big matmul
    # in f32 mode).
    hfT32 = nc.dram_tensor("hfT32", [H, R], F32, kind="ExternalInput")
    if mode == "f32r":
        hfT_m = nc.dram_tensor("hfT", [H, R], F32R, kind="ExternalInput")
        wv_m = nc.dram_tensor("wv", [H, VS], F32R, kind="ExternalInput")
    elif mode == "f16x3":
        hfT_m = nc.dram_tensor("hfT", [H, R], F16, kind="ExternalInput")
        hfT_lo = nc.dram_tensor("hfT_lo", [H, R], F16, kind="ExternalInput")
        wv_m = nc.dram_tensor("wv", [H, VS], F16, kind="ExternalInput")
        wv_lo = nc.dram_tensor("wv_lo", [H, VS], F16, kind="ExternalInput")
    else:
        hfT_m = hfT32
        wv_m = nc.dram_tensor("wv", [H, VS], F32, kind="ExternalInput")

    hfT_own = nc.dram_tensor("hfT_own", [H, RPC], F32, kind="ExternalInput")
    wp = nc.dram_tensor("wp", [H, 3], F32, kind="ExternalInput")
    bp_rep = nc.dram_tensor("bp_rep", [128, 3], F32, kind="ExternalInput")
    maskbias = nc.dram_tensor("maskbias", [128, 1], F32, kind="ExternalInput")
    srcT = nc.dram_tensor("srcT", [S, RPC], F32, kind="ExternalInput")
    smap = nc.dram_tensor("smap", [S, SDV], F32, kind="ExternalInput")
    tgtT = nc.dram_tensor("tgtT", [TDV, RPC], F32, kind="ExternalInput")
    tmap = nc.dram_tensor("tmap", [TDV, TDV], F32, kind="ExternalInput")

    # ---------------- outputs ----------------
    out_vocab = nc.dram_tensor("out_vocab", [R, VS], F32, kind="ExternalOutput")
    out_copy = nc.dram_tensor("out_copy", [RPC, SDV + TDV], F32,
                              kind="ExternalOutput")
    vmax_o = nc.dram_tensor("vmax", [R, 1], F32, kind="ExternalOutput")
    cmax_o = nc.dram_tensor("cmax", [RPC, 1], F32, kind="ExternalOutput")
    p_o = nc.dram_tensor("p_out", [R, 3], F32, kind="ExternalOutput")

    with tile.TileContext(nc) as tc:
        with (
            tc.tile_pool(name="wvp", bufs=1) as wvp,
            tc.tile_pool(name="hfp", bufs=3) as hfp,
            tc.tile_pool(name="ep", bufs=MT_PER_G + 1) as epool,
            tc.tile_pool(name="small", bufs=4) as smallp,
            tc.tile_pool(name="cst", bufs=1) as cstp,
            tc.tile_pool(name="accp", bufs=MT_PER_G + 1) as accp,
            tc.tile_pool(name="psA", bufs=2, space="PSUM") as psA,   # 2x3 banks
            tc.tile_pool(name="psB", bufs=2, space="PSUM") as psB,   # 2x1 bank
            tc.tile_pool(name="dram", bufs=2 * GROUPS + 2, space="DRAM") as dp,
        ):
            # --- dummy collective: absorbs core launch skew + CC init ---
            warm_in = dp.tile([1, 8], F32)
            warm_out = dp.tile([1, 8], F32)
            nc.gpsimd.collective_compute(
                "AllReduce", OP.add,
                replica_groups=[list(range(N_CORES))],
                ins=[warm_in[:].opt()], outs=[warm_out[:].opt()],
            )

            # --- resident weights / constants ---
            # wv loaded per k-tile so the first matmuls can start early
            wv_t = wvp.tile([128, KT, VS], mm_dt, tag="wv")
            wv_r = wv_m.rearrange("(kt p) v -> p kt v", p=128)
            for ki in range(KT):
                nc.sync.dma_start(wv_t[:, ki, :], wv_r[:, ki, :])
            if mode == "f16x3":
                wv_lo_t = wvp.tile([128, KT, VS], F16, tag="wvlo")
                wvl_r = wv_lo.rearrange("(kt p) v -> p kt v", p=128)
                for ki in range(KT):
                    nc.sync.dma_start(wv_lo_t[:, ki, :], wvl_r[:, ki, :])

            wp_t = cstp.tile([128, KT, 3], F32, tag="wp")
            nc.sync.dma_start(wp_t[:], wp.rearrange("(kt p) n -> p kt n", p=128))
            bp_t = cstp.tile([128, 3], F32, tag="bp")
            nc.sync.dma_start(bp_t[:], bp_rep[:, :])
            mask_t = cstp.tile([128, 1], F32, tag="mask")
            nc.sync.dma_start(mask_t[:], maskbias[:, :])

            # --- copy path (own 256 rows), exact fp32 ---
            srcT_t = cstp.tile([128, KT, RPC], F32, tag="srcT")
            nc.sync.dma_start(srcT_t[:], srcT.rearrange("(kt p) r -> p kt r", p=128))
            smap_t = cstp.tile([128, KT, SDV], F32, tag="smap")
            nc.sync.dma_start(smap_t[:], smap.rearrange("(kt p) v -> p kt v", p=128))
            tgtT_t = cstp.tile([128, KT, RPC], F32, tag="tgtT")
            nc.sync.dma_start(tgtT_t[:], tgtT.rearrange("(kt p) r -> p kt r", p=128))
            tmap_t = cstp.tile([128, KT, TDV], F32, tag="tmap")
            nc.sync.dma_start(tmap_t[:], tmap.rearrange("(kt p) v -> p kt v", p=128))
            hfo_t = cstp.tile([128, KT, RPC], F32, tag="hfo")
            nc.sync.dma_start(hfo_t[:], hfT_own.rearrange("(kt p) r -> p kt r", p=128))

            for mo in range(RPC // 128):  # 2 own m-tiles
                rs = slice(mo * 128, (mo + 1) * 128)
                # p for own rows (identical math to the big-loop p below)
                ps_p = psB.tile([128, 3], F32, tag="psB")
                for ki in range(KT):
                    nc.tensor.matmul(ps_p[:], hfo_t[:, ki, rs], wp_t[:, ki, :],
                                     start=(ki == 0), stop=(ki == KT - 1))
                nc.vector.tensor_add(ps_p[:], ps_p[:], bp_t[:])
                p_own = smallp.tile([128, 3], F32, tag="p_own")
                psum_own = smallp.tile([128, 1], F32, tag="psum_own")
                nc.scalar.activation(p_own[:], ps_p[:], AF.Exp,
                                     accum_out=psum_own[:])
                prcp = smallp.tile([128, 1], F32, tag="prcp")
                nc.vector.reciprocal(prcp[:], psum_own[:])
                nc.vector.tensor_scalar(p_own[:], p_own[:], prcp[:], None,
                                        op0=OP.mult)

                # copy-source block: (A_chunk @ smap) * p_cs
                ps_cs = psB.tile([128, SDV], F32, tag="psB")
                for ki in range(KT):
                    nc.tensor.matmul(ps_cs[:], srcT_t[:, ki, rs],
                                     smap_t[:, ki, :],
                                     start=(ki == 0), stop=(ki == KT - 1))
                o_cs = smallp.tile([128, SDV], F32, tag="o_cs")
                nc.vector.tensor_scalar(o_cs[:], ps_cs[:], p_own[:, 0:1], None,
                                        op0=OP.mult)
                # copy-target block: (Tattn_chunk @ tmap) * p_ct
                ps_ct = psB.tile([128, TDV], F32, tag="psB")
                for ki in range(KT):
                    nc.tensor.matmul(ps_ct[:], tgtT_t[:, ki, rs],
                                     tmap_t[:, ki, :],
                                     start=(ki == 0), stop=(ki == KT - 1))
                o_ct = smallp.tile([128, TDV], F32, tag="o_ct")
                nc.vector.tensor_scalar(o_ct[:], ps_ct[:], p_own[:, 1:2], None,
                                        op0=OP.mult)

                cmx = smallp.tile([128, 2], F32, tag="cmx")
                nc.vector.tensor_reduce(cmx[:, 0:1], o_cs[:], axis=AX.X, op=OP.max)
                nc.vector.tensor_reduce(cmx[:, 1:2], o_ct[:], axis=AX.X, op=OP.max)
                cmx1 = smallp.tile([128, 1], F32, tag="cmx1")
                nc.vector.tensor_reduce(cmx1[:], cmx[:], axis=AX.X, op=OP.max)

                nc.sync.dma_start(out_copy[rs, 0:SDV], o_cs[:])
                nc.sync.dma_start(out_copy[rs, SDV:SDV + TDV], o_ct[:])
                nc.sync.dma_start(cmax_o[rs, :], cmx1[:])

            # --- main vocab loop, grouped for pipelined allreduces ---
            vmax_all = cstp.tile([128, M_TILES], F32, tag="vmax_all")
            p_all = cstp.tile([128, M_TILES, 3], F32, tag="p_all")
            hfT32_r = hfT32.rearrange("(kt p) r -> p kt r", p=128)
            hfTm_r = hfT_m.rearrange("(kt p) r -> p kt r", p=128)
            if mode == "f16x3":
                hfTl_r = hfT_lo.rearrange("(kt p) r -> p kt r", p=128)
            pair_tiles = {}
            for g in range(GROUPS):
                sum_ts = []
                e_ts = []
                pg_ts = []
                for t_ in range(MT_PER_G):
                    mi = g * MT_PER_G + t_
                    ms = slice(mi * 128, (mi + 1) * 128)

                    pi = mi // 2
                    if pi not in pair_tiles:
                        prs = slice(pi * 256, (pi + 1) * 256)
                        hf32_p = hfp.tile([128, KT, 256], F32, tag="hf32")
                        nc.sync.dma_start(hf32_p[:], hfT32_r[:, :, prs])
                        if mode == "f32":
                            hfm_p, hfl_p = hf32_p, None
                        else:
                            hfm_p = hfp.tile([128, KT, 256], mm_dt, tag="hfm")
                            nc.sync.dma_start(hfm_p[:], hfTm_r[:, :, prs])
                            hfl_p = None
                            if mode == "f16x3":
                                hfl_p = hfp.tile([128, KT, 256], F16, tag="hfl")
                                nc.sync.dma_start(hfl_p[:], hfTl_r[:, :, prs])
                        pair_tiles = {pi: (hf32_p, hfm_p, hfl_p)}
                    hf32_f, hfm_f, hfl_f = pair_tiles[pi]
                    hs = slice((mi % 2) * 128, (mi % 2) * 128 + 128)

                    # p switch probs (exact f32) for this m-tile
                    ps_p = psB.tile([128, 3], F32, tag="psB")
                    for ki in range(KT):
                        nc.tensor.matmul(ps_p[:], hf32_f[:, ki, hs],
                                         wp_t[:, ki, :],
                                         start=(ki == 0), stop=(ki == KT - 1))
                    nc.vector.tensor_add(ps_p[:], ps_p[:], bp_t[:])
                    p_t = smallp.tile([128, 3], F32, tag="p_t")
                    psum_t = smallp.tile([128, 1], F32, tag="psum_t")
                    nc.scalar.activation(p_t[:], ps_p[:], AF.Exp,
                                         accum_out=psum_t[:])
                    prcp = smallp.tile([128, 1], F32, tag="prcp2")
                    nc.vector.reciprocal(prcp[:], psum_t[:])
                    nc.vector.tensor_scalar(p_all[:, mi, :], p_t[:], prcp[:],
                                            None, op0=OP.mult)
                    pg_ts.append(mi)

                    # big matmul in psum chunks; exp+rowsum fused per chunk
                    e_t = epool.tile([128, VS], F32, tag="e")
                    acc_t = accp.tile([128, 3], F32, tag="acc")
                    col = 0
                    for ci, cw in enumerate(CHUNKS):
                        ps_c = psA.tile([128, cw], F32, tag="psA")
                        for n0 in range(0, cw, 512):
                            nw = min(512, cw - n0)
                            for ki in range(KT):
                                first = ki == 0
                                last = ki == KT - 1
                                if mode == "f16x3":
                                    nc.tensor.matmul(
                                        ps_c[:, n0:n0 + nw], hfm_f[:, ki, hs],
                                        wv_t[:, ki, col + n0:col + n0 + nw],
                                        start=first, stop=False)
                                    nc.tensor.matmul(
                                        ps_c[:, n0:n0 + nw], hfm_f[:, ki, hs],
                                        wv_lo_t[:, ki, col + n0:col + n0 + nw],
                                        start=False, stop=False)
                                    nc.tensor.matmul(
                                        ps_c[:, n0:n0 + nw], hfl_f[:, ki, hs],
                                        wv_t[:, ki, col + n0:col + n0 + nw],
                                        start=False, stop=last)
                                else:
                                    nc.tensor.matmul(
                                        ps_c[:, n0:n0 + nw], hfm_f[:, ki, hs],
                                        wv_t[:, ki, col + n0:col + n0 + nw],
                                        start=first, stop=last)
                        if ci == 0:
                            # global vocab col 0 pad mask (-1e30 on core 0)
                            nc.vector.tensor_add(ps_c[:, 0:1], ps_c[:, 0:1],
                                                 mask_t[:])
                        nc.scalar.activation(e_t[:, col:col + cw], ps_c[:],
                                             AF.Exp,
                                             accum_out=acc_t[:, ci:ci + 1])
                        col += cw

                    sumloc = smallp.tile([128, 1], F32, tag="sumloc")
                    nc.vector.tensor_reduce(sumloc[:], acc_t[:], axis=AX.X,
                                            op=OP.add)
                    sum_ts.append(sumloc)
                    e_ts.append(e_t)

                # allreduce this group's row sums
                bin_ = dp.tile([MT_PER_G, 128], F32)
                bout = dp.tile([MT_PER_G, 128], F32)
                for t_ in range(MT_PER_G):
                    nc.sync.dma_start(bin_[t_:t_ + 1, :], sum_ts[t_][:])
                nc.gpsimd.collective_compute(
                    "AllReduce", OP.add,
                    replica_groups=[list(range(N_CORES))],
                    ins=[bin_[:].opt()], outs=[bout[:].opt()],
                )

                for t_ in range(MT_PER_G):
                    mi = g * MT_PER_G + t_
                    ms = slice(mi * 128, (mi + 1) * 128)
                    sf = smallp.tile([128, 1], F32, tag="sf")
                    nc.sync.dma_start(sf[:], bout[t_:t_ + 1, :])
                    rcp = smallp.tile([128, 1], F32, tag="rcp")
                    nc.vector.reciprocal(rcp[:], sf[:])
                    scale = smallp.tile([128, 1], F32, tag="scale")
                    nc.vector.tensor_mul(scale[:], rcp[:], p_all[:, mi, 2:3])

                    e_t = e_ts[t_]
                    if t_ % 2 == 0:
                        nc.vector.tensor_scalar(e_t[:], e_t[:], scale[:], None,
                                                op0=OP.mult)
                    else:
                        nc.scalar.activation(e_t[:], e_t[:], AF.Copy,
                                             scale=scale[:])
                    nc.vector.tensor_reduce(vmax_all[:, mi:mi + 1], e_t[:],
                                            axis=AX.X, op=OP.max)
                    nc.sync.dma_start(out_vocab[ms, :], e_t[:])

            # batched small outputs
            nc.sync.dma_start(
                vmax_o.rearrange("(mt p) o -> p mt o", p=128),
                vmax_all[:].unsqueeze(2))
            nc.sync.dma_start(
                p_o.rearrange("(mt p) c -> p mt c", p=128), p_all[:])

    return nc


def _get_kernel(mode: str):
    if mode not in _KERNEL_CACHE:
        _KERNEL_CACHE[mode] = _build(mode)
    return _KERNEL_CACHE[mode]


# ----------------------------------------------------------------------------
# Host side
# ----------------------------------------------------------------------------

def _f16_split(a: np.ndarray):
    hi = a.astype(np.float16)
    lo = (a - hi.astype(np.float32)).astype(np.float16)
    return hi, lo


def _prepare_inputs(mode, hiddens, Wp, bp, Wv, bv, source_attentions,
                    source_attention_maps, target_attentions,
                    target_attention_maps):
    hf = np.ascontiguousarray(hiddens.reshape(R, H))
    hfT = np.ascontiguousarray(hf.T)                       # (H, R) f32
    wp_f = np.ascontiguousarray(Wp.astype(np.float32))
    bp_rep = np.broadcast_to(bp.astype(np.float32), (128, 3)).copy()

    if mode == "f16x3":
        hfT_hi, hfT_lo = _f16_split(hfT)
        wv_hi, wv_lo = _f16_split(Wv)

    in_maps = []
    for c in range(N_CORES):
        b = c // (N_CORES // B)
        toff = (c % (N_CORES // B)) * RPC
        rs = slice(c * RPC, (c + 1) * RPC)
        vs = slice(c * VS, (c + 1) * VS)

        mb = np.zeros((128, 1), np.float32)
        if c == 0:
            mb[:] = NEG

        m = {
            "hfT32": hfT,
            "hfT_own": np.ascontiguousarray(hfT[:, rs]),
            "wp": wp_f,
            "bp_rep": bp_rep,
            "maskbias": mb,
            "srcT": np.ascontiguousarray(
                source_attentions[b, toff:toff + RPC, :].T),
            "smap": np.ascontiguousarray(source_attention_maps[b]),
            "tgtT": np.ascontiguousarray(
                target_attentions[b, toff:toff + RPC, :].T),
            "tmap": np.ascontiguousarray(target_attention_maps[b]),
        }
        if mode == "f32r":
            m["hfT"] = hfT
            m["wv"] = np.ascontiguousarray(Wv[:, vs])
        elif mode == "f16x3":
            m["hfT"] = hfT_hi
            m["hfT_lo"] = hfT_lo
            m["wv"] = np.ascontiguousarray(wv_hi[:, vs])
            m["wv_lo"] = np.ascontiguousarray(wv_lo[:, vs])
        else:
            m["wv"] = np.ascontiguousarray(Wv[:, vs])
        in_maps.append(m)
    return in_maps


def _assemble(results, bv):
    VT = V + SDV + TDV
    probs = np.empty((R, VT), np.float32)
    vmax = np.empty((R, N_CORES), np.float32)
    copy_blk = np.empty((R, SDV + TDV), np.float32)
    cmax = np.empty((R,), np.float32)
    for c in range(N_CORES):
        r = results[c]
        probs[:, c * VS:(c + 1) * VS] = r["out_vocab"]
        vmax[:, c] = r["vmax"][:, 0]
        rs = slice(c * RPC, (c + 1) * RPC)
        copy_blk[rs] = r["out_copy"]
        cmax[rs] = r["cmax"][:, 0]
    probs[:, V:] = copy_blk

    if np.any(bv):
        # setup_inputs always produces bv == 0; exact-general fallback for
        # nonzero bv recomputes the vocab renormalization on the host.
        p_out = results[0]["p_out"]
        p_gen = p_out[:, 2:3]
        e_rel = probs[:, :V] / np.where(p_gen == 0, 1.0, p_gen)
        e_rel = e_rel * np.exp(bv.astype(np.float32))[None, :]
        probs[:, :V] = e_rel / e_rel.sum(axis=1, keepdims=True) * p_gen
        vocab_max = probs[:, :V].max(axis=1)
        best = np.where(cmax >= vocab_max, 1, 0)
        preds = np.empty((R,), np.int64)
        for i in range(R):
            preds[i] = int(np.argmax(probs[i]))
        return probs, preds.astype(np.int32)

    # region winner per row: [vocab shard 0..7, copy]; first max wins so the
    # natural global-column order is preserved on exact ties.
    cand = np.concatenate([vmax, cmax[:, None]], axis=1)   # (R, 9)
    best = np.argmax(cand, axis=1)

    preds = np.empty((R,), np.int64)
    copy_idx = np.argmax(copy_blk, axis=1)                 # cheap, all rows
    for i in range(R):
        c = best[i]
        if c == N_CORES:
            preds[i] = V + copy_idx[i]
        else:
            preds[i] = c * VS + int(np.argmax(probs[i, c * VS:(c + 1) * VS]))
    return probs, preds


def kernel(hiddens, Wp, bp, Wv, bv, source_attentions, source_attention_maps,
           target_attentions, target_attention_maps):
    mode = MODE
    hiddens = np.asarray(hiddens, np.float32)
    Wp = np.asarray(Wp, np.float32)
    bp = np.asarray(bp, np.float32)
    Wv = np.asarray(Wv, np.float32)
    bv = np.asarray(bv, np.float32)
    source_attentions = np.asarray(source_attentions, np.float32)
    source_attention_maps = np.asarray(source_attention_maps, np.float32)
    target_attentions = np.asarray(target_attentions, np.float32)
    target_attention_maps = np.asarray(target_attention_maps, np.float32)

    nc = _get_kernel(mode)
    in_maps = _prepare_inputs(mode, hiddens, Wp, bp, Wv, bv,
                              source_attentions, source_attention_maps,
                              target_attentions, target_attention_maps)
    res = run_bass_kernel_spmd(
        nc, in_maps, core_ids=list(range(N_CORES)),
        trace=bool(int(os.environ.get("PG_TRACE", "0"))),
        tmpdir=os.environ.get("PG_TMPDIR"),
    )
    kernel.last_result = res

    probs, preds = _assemble(res.results, bv)
    probs = probs.reshape(B, T, V + SDV + TDV)
    preds = np.asarray(preds, np.int32).reshape(B, T)
    return probs, preds
